# revision 12
# baseline (speedup 1.0000x reference)
"""Causal self-attention (B=2, T=2048, C=1024, NH=16, HD=64) on 8 TRN2 cores.

Sharding: TP over heads x DP over batch. Core i handles batch i//4 and
heads [4*(i%4) .. 4*(i%4)+4). Each core:
  1. QK^T projection in transposed layout: qkT[c_out, t] = w_qk.T @ x.T
     (lhsT = w_qk natural, rhs = xT; Q^T/K^T land with head_dim on
     partitions, which feeds the S matmul directly).
  2. V projection in natural layout [t, d] (lhsT = xT tiles), augmented
     with a ones column per head so the PV matmul also produces the
     softmax denominators for free.
  3. Flash-style causal attention per head in S^T = [k, q] layout:
     S^T = K^T.T @ Q^T (two heads row-packed in the 128x128 PE array via
     tile_position), exp via ScalarE (scale=1/8 fused), causal mask on
     the diagonal 128x128 block only, PV matmul accumulating y_aug^T
     over k-tiles, then normalize by the broadcast reciprocal row.
  4. Output projection partial [t, 1024] = y.T @ w_proj_shard (+bias on
     rank-0 core of each group via a K=1 ones matmul).
  5. ReduceScatter(add) over the 4 cores of the batch group; core gets
     its 512-row slice of the summed projection.
Host assembles the 8 slices into [B, T, C].

Compute dtype bf16 (fp32 PSUM accumulation everywhere).
"""
import sys
import types

import numpy as np
import ml_dtypes

import concourse.bass as bass
import concourse.bacc as bacc
import concourse.tile as tile
import concourse.mybir as mybir
from concourse.bass_utils import run_bass_kernel_spmd

B, T, C, NH, HD = 2, 2048, 1024, 16, 64
N_CORES = 8
TP, DP = 4, 2
HLOC = NH // TP            # 4 heads per core
DLOC = HLOC * HD           # 256
GROUPS = [[0, 1, 2, 3], [4, 5, 6, 7]]
NKT = T // 128             # 16 k-tiles / t-tiles
NCT = C // 128             # 8 c_in tiles
NQC = T // 512             # 4 q-chunks of 512
TSLICE = T // TP           # 512 output rows per core

F32 = mybir.dt.float32
BF16 = mybir.dt.bfloat16
AF = mybir.ActivationFunctionType
BF16_NP = ml_dtypes.bfloat16

_CACHED_NC = None
RECIP_MODE = "approx"  # set after smoke validation: approx | exact | lnexp


def _install_ntff_hook():
    """Register the axon NTFF profiling shim if the image lacks it."""
    if "antenv.axon_hooks" in sys.modules:
        return
    try:
        from trn_agent_boot.trn_boot import _ntff_profile_via_ctypes
        hook = _ntff_profile_via_ctypes("/opt/axon/libaxon_pjrt.so")
        import antenv
        mod = types.ModuleType("antenv.axon_hooks")
        mod.get_axon_ntff_profile_hook = lambda: hook
        mod.set_axon_ntff_profile_hook = lambda h: None
        sys.modules["antenv.axon_hooks"] = mod
        antenv.axon_hooks = mod
    except Exception:
        pass


def _span(ki):
    """q-span length for k-tile ki under causality (q >= 128*ki)."""
    return T - 128 * ki


def build_kernel_body(nc, tc, es, d):
    ES = 512  # eviction / moving chunk

    sbuf = es.enter_context(tc.tile_pool(name="sbuf", bufs=1))
    sbuf2 = es.enter_context(tc.tile_pool(name="sbuf2", bufs=2))
    spool = es.enter_context(tc.tile_pool(name="spool", bufs=4, space="PSUM"))
    mpool = es.enter_context(tc.tile_pool(name="mpool", bufs=2, space="PSUM"))
    ypool = es.enter_context(tc.tile_pool(name="ypool", bufs=2, space="PSUM"))
    dram = es.enter_context(tc.tile_pool(name="dram", bufs=1, space="DRAM"))

    # ---- constant / input tiles -------------------------------------------
    xT = [sbuf.tile([128, T], BF16, tag=f"xT{c}", name=f"xT{c}") for c in range(NCT)]
    for c in range(NCT):
        for tc4 in range(NQC):
            nc.sync.dma_start(
                xT[c][:, tc4 * ES:(tc4 + 1) * ES],
                d["xT"][c * 128:(c + 1) * 128, tc4 * ES:(tc4 + 1) * ES])

    wqk = [sbuf.tile([128, 4 * 128], BF16, tag=f"wqk{c}", name=f"wqk{c}") for c in range(NCT)]
    for c in range(NCT):
        nc.sync.dma_start(wqk[c][:], d["w_qk"][c * 128:(c + 1) * 128, :])
    wv = [sbuf.tile([128, DLOC], BF16, tag=f"wv{c}", name=f"wv{c}") for c in range(NCT)]
    for c in range(NCT):
        nc.sync.dma_start(wv[c][:], d["w_v"][c * 128:(c + 1) * 128, :])
    wp = [sbuf.tile([128, C], BF16, tag=f"wp{p}", name=f"wp{p}") for p in range(2)]
    for p in range(2):
        nc.sync.dma_start(wp[p][:], d["w_proj"][p * 128:(p + 1) * 128, :])

    bqk = [sbuf.tile([128, 1], F32, tag=f"bqk{m}", name=f"bqk{m}") for m in range(4)]
    for m in range(4):
        nc.sync.dma_start(bqk[m][:], d["b_qk"][m * 128:(m + 1) * 128, :])
    bv = sbuf.tile([1, DLOC], BF16, tag="bv")
    nc.sync.dma_start(bv[:], d["b_v"][:])
    bp = sbuf.tile([1, C], BF16, tag="bp")
    nc.sync.dma_start(bp[:], d["b_proj"][:])
    mask = sbuf.tile([128, 128], BF16, tag="mask")
    nc.sync.dma_start(mask[:], d["mask"][:])
    ones1 = sbuf.tile([1, 128], BF16, tag="ones1")
    nc.vector.memset(ones1[:], 1.0)
    ones1f = sbuf.tile([1, 64], F32, tag="ones1f")
    nc.vector.memset(ones1f[:], 1.0)

    # ---- QK^T projection (transposed layout) ------------------------------
    # m-tile order q01, k01, q23, k23 so pair-0 attention can start early.
    # qkT[m]: [128, T] bf16; m=0: Q^T heads 0,1  m=1: K^T heads 0,1
    #                        m=2: Q^T heads 2,3  m=3: K^T heads 2,3
    qkT = [sbuf.tile([128, T], BF16, tag=f"qkT{m}", name=f"qkT{m}") for m in range(4)]
    # w_qk column order (host side): [q01 | q23 | k01 | k23] blocks of 128.
    wcol_of_m = [0, 2, 1, 3]  # m-tile -> w_qk column block

    def emit_qk(m):
        wc = wcol_of_m[m]
        for tc4 in range(NQC):
            ps = mpool.tile([128, ES], F32, tag="mm")
            for c in range(NCT):
                nc.tensor.matmul(
                    ps[:], wqk[c][:, wc * 128:(wc + 1) * 128],
                    xT[c][:, tc4 * ES:(tc4 + 1) * ES],
                    start=(c == 0), stop=(c == NCT - 1))
            nc.vector.tensor_scalar_add(
                qkT[m][:, tc4 * ES:(tc4 + 1) * ES], ps[:], bqk[wc][:])

    # ---- V projection (natural layout, ones-augmented) --------------------
    # vsb[tt]: [128, 4*65]; per head h: cols h*65..h*65+63 = V_h, col h*65+64 = 1
    vsb = [sbuf.tile([128, HLOC * (HD + 1)], BF16, tag=f"vsb{tt}", name=f"vsb{tt}")
           for tt in range(NKT)]

    def emit_v(tt):
        ps = mpool.tile([128, DLOC], F32, tag="mm")
        for c in range(NCT):
            nc.tensor.matmul(
                ps[:], xT[c][:, tt * 128:(tt + 1) * 128], wv[c][:],
                start=(c == 0), stop=False)
        nc.tensor.matmul(ps[:], ones1[:, 0:128], bv[:], start=False, stop=True)
        vgrp = vsb[tt][:].rearrange("p (h x) -> p h x", h=HLOC)
        nc.vector.tensor_copy(
            vgrp[:, :, 0:HD],
            ps[:].rearrange("p (h x) -> p h x", h=HLOC))
        nc.vector.memset(vgrp[:, :, HD:HD + 1], 1.0)

    emit_qk(0)
    emit_qk(1)
    for tt in range(NKT):
        emit_v(tt)
    emit_qk(2)
    emit_qk(3)

    # ---- attention per head pair ------------------------------------------
    # y_norm[p]: [128, T] bf16; partitions 0:64 head 2p, 64:128 head 2p+1
    yn = [sbuf.tile([128, T], BF16, tag=f"yn{p}", name=f"yn{p}") for p in range(2)]

    for p in range(2):
        qt, kt = qkT[2 * p], qkT[2 * p + 1]
        # P^T tiles for this pair: one per (ki, h), span T-128*ki
        pt = [[sbuf.tile([128, _span(ki)], BF16, tag=f"P{ki}h{h}", name=f"P{ki}h{h}")
               for h in range(2)] for ki in range(NKT)]
        for ki in range(NKT):
            q0 = 128 * ki           # first valid q for this k-tile
            for h in range(2):
                hp = 64 * h         # partition offset of head in qt/kt tiles
                lhs = kt[hp:hp + 64, ki * 128:(ki + 1) * 128]
                off = 0
                while off < _span(ki):
                    n = min(ES, _span(ki) - off)
                    ps = spool.tile([128, ES], F32, tag="S")
                    nc.tensor.matmul(
                        ps[:, 0:n], lhs,
                        qt[hp:hp + 64, q0 + off:q0 + off + n],
                        start=True, stop=True, tile_position=(hp, 0))
                    nc.scalar.activation(
                        pt[ki][h][:, off:off + n], ps[:, 0:n], AF.Exp,
                        scale=0.125)
                    off += n
                # causal mask on the diagonal 128x128 block
                nc.vector.tensor_mul(
                    pt[ki][h][:, 0:128], pt[ki][h][:, 0:128], mask[:])
            # PV for completed q-chunks
            if ki % 4 == 3:
                qc = ki // 4
                qlo = qc * ES
                for h in range(2):
                    head = 2 * p + h
                    yps = ypool.tile([65, ES], F32, tag="y")
                    nkj = 4 * qc + 4
                    for kj in range(nkj):
                        rel = qlo - 128 * kj   # chunk start within P span
                        if rel >= 0:
                            rhs = pt[kj][h][:, rel:rel + ES]
                            out = yps[:, 0:ES]
                        else:
                            rhs = pt[kj][h][:, 0:ES + rel]
                            out = yps[:, -rel:ES]
                        nc.tensor.matmul(
                            out, vsb[kj][:, head * 65:head * 65 + 65], rhs,
                            start=(kj == 0), stop=(kj == nkj - 1))
                    srow = sbuf2.tile([1, ES], F32, tag="srow", name="srow")
                    nc.vector.tensor_copy(srow[:], yps[64:65, :])
                    rec = sbuf2.tile([1, ES], F32, tag="rec", name="rec")
                    nc.vector.reciprocal_approx_fast(rec[:], srow[:])
                    # broadcast 1/s across 64 partitions via a K=1 matmul
                    bcps = mpool.tile([64, ES], F32, tag="mm", name="bcps")
                    nc.tensor.matmul(bcps[:], ones1f[:], rec[:],
                                     start=True, stop=True)
                    bc = sbuf2.tile([64, ES], F32, tag="bc", name="bc")
                    nc.scalar.copy(bc[:], bcps[:])
                    nc.vector.tensor_mul(
                        yn[p][64 * h:64 * h + 64, qlo:qlo + ES],
                        yps[0:64, :], bc[:])

    # ---- output projection + bias, straight to DRAM -----------------------
    partial = dram.tile([T, C], F32)
    for tt in range(NKT):
        for cc in range(2):
            ps = mpool.tile([128, ES], F32, tag="mm")
            nc.tensor.matmul(
                ps[:], yn[0][:, tt * 128:(tt + 1) * 128],
                wp[0][:, cc * ES:(cc + 1) * ES], start=True, stop=False)
            nc.tensor.matmul(
                ps[:], yn[1][:, tt * 128:(tt + 1) * 128],
                wp[1][:, cc * ES:(cc + 1) * ES], start=False, stop=False)
            nc.tensor.matmul(
                ps[:], ones1[:], bp[:, cc * ES:(cc + 1) * ES],
                start=False, stop=True)
            po = sbuf2.tile([128, ES], F32, tag="pout", name="pout")
            nc.vector.tensor_copy(po[:], ps[:])
            nc.sync.dma_start(
                partial[tt * 128:(tt + 1) * 128, cc * ES:(cc + 1) * ES],
                po[:])

    # ---- ReduceScatter over the batch group -------------------------------
    nc.gpsimd.collective_compute(
        "ReduceScatter",
        mybir.AluOpType.add,
        replica_groups=GROUPS,
        ins=[partial.opt()],
        outs=[d["rs_buf"].opt()],
    )
    nc.sync.dma_start(d["out"][:], d["rs_buf"][:])


def build_nc():
    global _CACHED_NC
    if _CACHED_NC is not None:
        return _CACHED_NC
    nc = bacc.Bacc("TRN2", target_bir_lowering=False, debug=False,
                   num_devices=N_CORES)
    d = {
        "xT": nc.dram_tensor("xT", [C, T], BF16, kind="ExternalInput").ap(),
        "w_qk": nc.dram_tensor("w_qk", [C, 2 * DLOC], BF16,
                               kind="ExternalInput").ap(),
        "w_v": nc.dram_tensor("w_v", [C, DLOC], BF16,
                              kind="ExternalInput").ap(),
        "b_qk": nc.dram_tensor("b_qk", [2 * DLOC, 1], F32,
                               kind="ExternalInput").ap(),
        "b_v": nc.dram_tensor("b_v", [1, DLOC], BF16,
                              kind="ExternalInput").ap(),
        "w_proj": nc.dram_tensor("w_proj", [DLOC, C], BF16,
                                 kind="ExternalInput").ap(),
        "b_proj": nc.dram_tensor("b_proj", [1, C], BF16,
                                 kind="ExternalInput").ap(),
        "mask": nc.dram_tensor("mask", [128, 128], BF16,
                               kind="ExternalInput").ap(),
        "out": nc.dram_tensor("out", [TSLICE, C], F32,
                              kind="ExternalOutput").ap(),
        "rs_buf": nc.dram_tensor("rs_buf", [TSLICE, C], F32).ap(),
    }
    from contextlib import ExitStack
    with tile.TileContext(nc) as tc, ExitStack() as es:
        build_kernel_body(nc, tc, es, d)
    nc.compile()
    _CACHED_NC = nc
    return nc


def make_in_maps(x, w_attn, b_attn, w_proj, b_proj):
    x = np.asarray(x, dtype=np.float32)
    w_attn = np.asarray(w_attn, dtype=np.float32)
    b_attn = np.asarray(b_attn, dtype=np.float32)
    w_proj = np.asarray(w_proj, dtype=np.float32)
    b_proj = np.asarray(b_proj, dtype=np.float32)

    # causal mask for the S^T-layout diagonal block: valid iff q >= k
    kr = np.arange(128)
    mask = (kr[None, :] >= kr[:, None]).astype(BF16_NP)  # [k,q]

    in_maps = []
    for i in range(N_CORES):
        b = i // TP
        g = i % TP
        heads = list(range(HLOC * g, HLOC * g + HLOC))
        qcols = np.concatenate(
            [np.arange(h * HD, (h + 1) * HD) for h in heads])
        kcols = qcols + C
        vcols = qcols + 2 * C
        # w_qk column blocks: [q01 | q23 | k01 | k23] (128 cols each)
        w_qk = np.concatenate(
            [w_attn[:, qcols], w_attn[:, kcols]], axis=1)
        b_qk = np.concatenate([b_attn[qcols], b_attn[kcols]])
        xT = np.ascontiguousarray(x[b].T)
        in_maps.append({
            "xT": xT.astype(BF16_NP),
            "w_qk": w_qk.astype(BF16_NP),
            "w_v": w_attn[:, vcols].astype(BF16_NP),
            "b_qk": b_qk[:, None].astype(np.float32),
            "b_v": b_attn[vcols][None, :].astype(BF16_NP),
            "w_proj": w_proj[qcols, :].astype(BF16_NP),
            "b_proj": (b_proj[None, :] if g == 0
                       else np.zeros((1, C), np.float32)).astype(BF16_NP),
            "mask": mask,
        })
    return in_maps


def run(x, w_attn, b_attn, w_proj, b_proj, trace=False):
    _install_ntff_hook()
    nc = build_nc()
    in_maps = make_in_maps(x, w_attn, b_attn, w_proj, b_proj)
    res = run_bass_kernel_spmd(nc, in_maps, list(range(N_CORES)), trace=trace)
    out = np.empty((B, T, C), dtype=np.float32)
    for i in range(N_CORES):
        b = i // TP
        g = i % TP
        out[b, g * TSLICE:(g + 1) * TSLICE, :] = res.results[i]["out"]
    return out, res


def kernel(x, w_attn, b_attn, w_proj, b_proj):
    out, _ = run(x, w_attn, b_attn, w_proj, b_proj, trace=False)
    return out


# revision 19
# speedup vs baseline: 1.4281x; 1.4281x over previous
"""Causal self-attention (B=2, T=2048, C=1024, NH=16, HD=64) on 8 TRN2 cores.

Sharding: TP over heads x DP over batch. Core i handles batch i//4 and
heads [4*(i%4) .. 4*(i%4)+4). Each core:
  1. QK^T projection in transposed layout: qkT[c_out, t] = w_qk.T @ x.T
     (lhsT = w_qk natural, rhs = xT; Q^T/K^T land with head_dim on
     partitions, which feeds the S matmul directly).
  2. V projection in natural layout [t, d] (lhsT = xT tiles), augmented
     with a ones column per head so the PV matmul also produces the
     softmax denominators for free.
  3. Flash-style causal attention per head in S^T = [k, q] layout:
     S^T = K^T.T @ Q^T (two heads row-packed in the 128x128 PE array via
     tile_position), exp via ScalarE (scale=1/8 fused), causal mask on
     the diagonal 128x128 block only, PV matmul accumulating y_aug^T
     over k-tiles, then normalize by the broadcast reciprocal row.
  4. Output projection partial [t, 1024] = y.T @ w_proj_shard (+bias on
     rank-0 core of each group via a K=1 ones matmul).
  5. ReduceScatter(add) over the 4 cores of the batch group; core gets
     its 512-row slice of the summed projection.
Host assembles the 8 slices into [B, T, C].

Compute dtype bf16 (fp32 PSUM accumulation everywhere).
"""
import sys
import types

import numpy as np
import ml_dtypes

import concourse.bass as bass
import concourse.bacc as bacc
import concourse.tile as tile
import concourse.mybir as mybir
from concourse.bass_utils import run_bass_kernel_spmd

B, T, C, NH, HD = 2, 2048, 1024, 16, 64
N_CORES = 8
TP, DP = 4, 2
HLOC = NH // TP            # 4 heads per core
DLOC = HLOC * HD           # 256
GROUPS = [[0, 1, 2, 3], [4, 5, 6, 7]]
NKT = T // 128             # 16 k-tiles / t-tiles
NCT = C // 128             # 8 c_in tiles
NQC = T // 512             # 4 q-chunks of 512
TSLICE = T // TP           # 512 output rows per core

F32 = mybir.dt.float32
F16 = mybir.dt.float16
BF16 = mybir.dt.bfloat16
AF = mybir.ActivationFunctionType
BF16_NP = ml_dtypes.bfloat16

_CACHED_NC = None
RECIP_MODE = "approx"  # set after smoke validation: approx | exact | lnexp


def _install_ntff_hook():
    """Register the axon NTFF profiling shim if the image lacks it."""
    if "antenv.axon_hooks" in sys.modules:
        return
    try:
        from trn_agent_boot.trn_boot import _ntff_profile_via_ctypes
        hook = _ntff_profile_via_ctypes("/opt/axon/libaxon_pjrt.so")
        import antenv
        mod = types.ModuleType("antenv.axon_hooks")
        mod.get_axon_ntff_profile_hook = lambda: hook
        mod.set_axon_ntff_profile_hook = lambda h: None
        sys.modules["antenv.axon_hooks"] = mod
        antenv.axon_hooks = mod
    except Exception:
        pass


def _span(ki):
    """q-span length for k-tile ki under causality (q >= 128*ki)."""
    return T - 128 * ki


def build_kernel_body(nc, tc, es, d):
    ES = 512  # eviction / moving chunk

    sbuf = es.enter_context(tc.tile_pool(name="sbuf", bufs=1))
    sbuf2 = es.enter_context(tc.tile_pool(name="sbuf2", bufs=2))
    spool = es.enter_context(tc.tile_pool(name="spool", bufs=4, space="PSUM"))
    mpool = es.enter_context(tc.tile_pool(name="mpool", bufs=2, space="PSUM"))
    ypool = es.enter_context(tc.tile_pool(name="ypool", bufs=2, space="PSUM"))
    dram = es.enter_context(tc.tile_pool(name="dram", bufs=1, space="DRAM"))

    # ---- constant / input tiles -------------------------------------------
    # chunk-major DMA order: the first QK matmul group needs all 8 c-tiles
    # of t-chunk 0, so land those 8 transfers first.
    xT = [sbuf.tile([128, T], BF16, tag=f"xT{c}", name=f"xT{c}") for c in range(NCT)]
    for tc4 in range(NQC):
        for c in range(NCT):
            nc.sync.dma_start(
                xT[c][:, tc4 * ES:(tc4 + 1) * ES],
                d["xT"][c * 128:(c + 1) * 128, tc4 * ES:(tc4 + 1) * ES])

    wqk = [sbuf.tile([128, 4 * 128], BF16, tag=f"wqk{c}", name=f"wqk{c}") for c in range(NCT)]
    for c in range(NCT):
        nc.sync.dma_start(wqk[c][:], d["w_qk"][c * 128:(c + 1) * 128, :])
    wv = [sbuf.tile([128, DLOC], BF16, tag=f"wv{c}", name=f"wv{c}") for c in range(NCT)]
    for c in range(NCT):
        nc.sync.dma_start(wv[c][:], d["w_v"][c * 128:(c + 1) * 128, :])
    wp = [sbuf.tile([128, C], BF16, tag=f"wp{p}", name=f"wp{p}") for p in range(2)]
    for p in range(2):
        nc.sync.dma_start(wp[p][:], d["w_proj"][p * 128:(p + 1) * 128, :])

    bqk = [sbuf.tile([128, 1], F32, tag=f"bqk{m}", name=f"bqk{m}") for m in range(4)]
    for m in range(4):
        nc.sync.dma_start(bqk[m][:], d["b_qk"][m * 128:(m + 1) * 128, :])
    bv = sbuf.tile([1, DLOC], BF16, tag="bv")
    nc.sync.dma_start(bv[:], d["b_v"][:])
    bp = sbuf.tile([1, C], BF16, tag="bp")
    nc.sync.dma_start(bp[:], d["b_proj"][:])
    mask = sbuf.tile([128, 128], BF16, tag="mask")
    nc.sync.dma_start(mask[:], d["mask"][:])
    ones1 = sbuf.tile([1, 128], BF16, tag="ones1")
    nc.vector.memset(ones1[:], 1.0)
    ones16 = sbuf.tile([1, 64], F16, tag="ones16")
    nc.vector.memset(ones16[:], 1.0)
    # broadcast the (rank-masked) projection bias across partitions once
    bpb = sbuf.tile([128, C], F32, tag="bpb")
    for cc in range(2):
        bps = mpool.tile([128, ES], F32, tag="mm", name="bps")
        nc.tensor.matmul(bps[:], ones1[:], bp[:, cc * ES:(cc + 1) * ES],
                         start=True, stop=True)
        nc.scalar.copy(bpb[:, cc * ES:(cc + 1) * ES], bps[:])

    # ---- QK^T projection (transposed layout) ------------------------------
    # m-tile order q01, k01, q23, k23 so pair-0 attention can start early.
    # qkT[m]: [128, T] bf16; m=0: Q^T heads 0,1  m=1: K^T heads 0,1
    #                        m=2: Q^T heads 2,3  m=3: K^T heads 2,3
    qkT = [sbuf.tile([128, T], BF16, tag=f"qkT{m}", name=f"qkT{m}") for m in range(4)]
    # w_qk column order (host side): [q01 | q23 | k01 | k23] blocks of 128.
    wcol_of_m = [0, 2, 1, 3]  # m-tile -> w_qk column block

    def emit_qk(m):
        wc = wcol_of_m[m]
        for tc4 in range(NQC):
            ps = mpool.tile([128, ES], F32, tag="mm")
            for c in range(NCT):
                nc.tensor.matmul(
                    ps[:], wqk[c][:, wc * 128:(wc + 1) * 128],
                    xT[c][:, tc4 * ES:(tc4 + 1) * ES],
                    start=(c == 0), stop=(c == NCT - 1))
            nc.vector.tensor_scalar_add(
                qkT[m][:, tc4 * ES:(tc4 + 1) * ES], ps[:], bqk[wc][:])

    # ---- V projection (natural layout, ones-augmented) --------------------
    # vsb[tt]: [128, 4*65]; per head h: cols h*65..h*65+63 = V_h, col h*65+64 = 1
    vsb = [sbuf.tile([128, HLOC * (HD + 1)], BF16, tag=f"vsb{tt}", name=f"vsb{tt}")
           for tt in range(NKT)]

    def emit_v(tt):
        ps = mpool.tile([128, DLOC], F32, tag="mm")
        for c in range(NCT):
            nc.tensor.matmul(
                ps[:], xT[c][:, tt * 128:(tt + 1) * 128], wv[c][:],
                start=(c == 0), stop=False)
        nc.tensor.matmul(ps[:], ones1[:, 0:128], bv[:], start=False, stop=True)
        vgrp = vsb[tt][:].rearrange("p (h x) -> p h x", h=HLOC)
        nc.vector.tensor_copy(
            vgrp[:, :, 0:HD],
            ps[:].rearrange("p (h x) -> p h x", h=HLOC))
        nc.vector.memset(vgrp[:, :, HD:HD + 1], 1.0)

    emit_qk(0)
    emit_qk(1)
    for tt in range(NKT):
        emit_v(tt)
    emit_qk(2)
    emit_qk(3)

    # ---- attention, q-chunk-outer, fused with projection + chunked RS -----
    # For each 512-wide q-chunk qc: compute S^T/exp/PV/normalize for all 4
    # heads over the causal k range, then project this chunk and kick off
    # its ReduceScatter while the next chunk computes.
    # y_norm[p]: [128, T] bf16; partitions 0:64 head 2p, 64:128 head 2p+1
    yn = [sbuf.tile([128, T], BF16, tag=f"yn{p}", name=f"yn{p}") for p in range(2)]
    partial = dram.tile([T, C], F16)

    def width(qc, ki):
        return ES - max(0, 128 * ki - ES * qc)

    for qc in range(NQC):
        nki = 4 * qc + 4
        poff = [0]
        for ki in range(nki):
            poff.append(poff[-1] + width(qc, ki))
        totw = poff[-1]
        for p in range(2):
            qt, kt = qkT[2 * p], qkT[2 * p + 1]
            ptile = [sbuf2.tile([128, totw], BF16, tag=f"P{h}", name=f"P{h}")
                     for h in range(2)]
            for ki in range(nki):
                w = width(qc, ki)
                qstart = max(ES * qc, 128 * ki)
                for h in range(2):
                    hp = 64 * h
                    ps = spool.tile([128, ES], F32, tag="S")
                    nc.tensor.matmul(
                        ps[:, 0:w],
                        kt[hp:hp + 64, ki * 128:(ki + 1) * 128],
                        qt[hp:hp + 64, qstart:qstart + w],
                        start=True, stop=True, tile_position=(hp, 0))
                    nc.scalar.activation(
                        ptile[h][:, poff[ki]:poff[ki] + w], ps[:, 0:w],
                        AF.Exp, scale=0.125)
                    if 128 * ki >= ES * qc:
                        # piece starts at the diagonal: mask its first block
                        nc.vector.tensor_mul(
                            ptile[h][:, poff[ki]:poff[ki] + 128],
                            ptile[h][:, poff[ki]:poff[ki] + 128], mask[:])
            for h in range(2):
                head = 2 * p + h
                yps = ypool.tile([65, ES], F32, tag="y")
                for ki in range(nki):
                    w = width(qc, ki)
                    nc.tensor.matmul(
                        yps[:, ES - w:ES],
                        vsb[ki][:, head * 65:head * 65 + 65],
                        ptile[h][:, poff[ki]:poff[ki] + w],
                        start=(ki == 0), stop=(ki == nki - 1))
                srow = sbuf2.tile([1, ES], F32, tag="srow", name="srow")
                nc.vector.tensor_copy(srow[:], yps[64:65, :])
                rec = sbuf2.tile([1, ES], F32, tag="rec", name="rec")
                nc.vector.reciprocal_approx_fast(rec[:], srow[:])
                rec16 = sbuf2.tile([1, ES], F16, tag="rec16", name="rec16")
                nc.scalar.copy(rec16[:], rec[:])
                # broadcast 1/s across 64 partitions via a fp16 K=1 matmul
                bcps = mpool.tile([64, ES], F32, tag="mm", name="bcps")
                nc.tensor.matmul(bcps[:], ones16[:], rec16[:],
                                 start=True, stop=True)
                bc = sbuf2.tile([64, ES], F32, tag="bc", name="bc")
                nc.scalar.copy(bc[:], bcps[:])
                nc.vector.tensor_mul(
                    yn[p][64 * h:64 * h + 64, qc * ES:(qc + 1) * ES],
                    yps[0:64, :], bc[:])

        # ---- projection of this q-chunk + bias, then its ReduceScatter ----
        for tt in range(4 * qc, 4 * qc + 4):
            for cc in range(2):
                ps = mpool.tile([128, ES], F32, tag="mm")
                nc.tensor.matmul(
                    ps[:], yn[0][:, tt * 128:(tt + 1) * 128],
                    wp[0][:, cc * ES:(cc + 1) * ES], start=True, stop=False)
                nc.tensor.matmul(
                    ps[:], yn[1][:, tt * 128:(tt + 1) * 128],
                    wp[1][:, cc * ES:(cc + 1) * ES], start=False, stop=True)
                po = sbuf2.tile([128, ES], F16, tag="pout", name="pout")
                nc.vector.tensor_add(po[:], ps[:], bpb[:, cc * ES:(cc + 1) * ES])
                nc.sync.dma_start(
                    partial[tt * 128:(tt + 1) * 128, cc * ES:(cc + 1) * ES],
                    po[:])
        nc.gpsimd.collective_compute(
            "ReduceScatter",
            mybir.AluOpType.add,
            replica_groups=GROUPS,
            ins=[partial[qc * 512:(qc + 1) * 512, :].opt()],
            outs=[d["rs_buf"][qc * 128:(qc + 1) * 128, :].opt()],
        )
        nc.gpsimd.dma_start(
            d["out"][qc * 128:(qc + 1) * 128, :],
            d["rs_buf"][qc * 128:(qc + 1) * 128, :])


def build_nc():
    global _CACHED_NC
    if _CACHED_NC is not None:
        return _CACHED_NC
    nc = bacc.Bacc("TRN2", target_bir_lowering=False, debug=False,
                   num_devices=N_CORES)
    d = {
        "xT": nc.dram_tensor("xT", [C, T], BF16, kind="ExternalInput").ap(),
        "w_qk": nc.dram_tensor("w_qk", [C, 2 * DLOC], BF16,
                               kind="ExternalInput").ap(),
        "w_v": nc.dram_tensor("w_v", [C, DLOC], BF16,
                              kind="ExternalInput").ap(),
        "b_qk": nc.dram_tensor("b_qk", [2 * DLOC, 1], F32,
                               kind="ExternalInput").ap(),
        "b_v": nc.dram_tensor("b_v", [1, DLOC], BF16,
                              kind="ExternalInput").ap(),
        "w_proj": nc.dram_tensor("w_proj", [DLOC, C], BF16,
                                 kind="ExternalInput").ap(),
        "b_proj": nc.dram_tensor("b_proj", [1, C], BF16,
                                 kind="ExternalInput").ap(),
        "mask": nc.dram_tensor("mask", [128, 128], BF16,
                               kind="ExternalInput").ap(),
        "out": nc.dram_tensor("out", [TSLICE, C], F16,
                              kind="ExternalOutput").ap(),
        "rs_buf": nc.dram_tensor("rs_buf", [TSLICE, C], F16).ap(),
    }
    from contextlib import ExitStack
    with tile.TileContext(nc) as tc, ExitStack() as es:
        build_kernel_body(nc, tc, es, d)
    nc.compile()
    _CACHED_NC = nc
    return nc


def make_in_maps(x, w_attn, b_attn, w_proj, b_proj):
    x = np.asarray(x, dtype=np.float32)
    w_attn = np.asarray(w_attn, dtype=np.float32)
    b_attn = np.asarray(b_attn, dtype=np.float32)
    w_proj = np.asarray(w_proj, dtype=np.float32)
    b_proj = np.asarray(b_proj, dtype=np.float32)

    # causal mask for the S^T-layout diagonal block: valid iff q >= k
    kr = np.arange(128)
    mask = (kr[None, :] >= kr[:, None]).astype(BF16_NP)  # [k,q]

    in_maps = []
    for i in range(N_CORES):
        b = i // TP
        g = i % TP
        heads = list(range(HLOC * g, HLOC * g + HLOC))
        qcols = np.concatenate(
            [np.arange(h * HD, (h + 1) * HD) for h in heads])
        kcols = qcols + C
        vcols = qcols + 2 * C
        # w_qk column blocks: [q01 | q23 | k01 | k23] (128 cols each)
        w_qk = np.concatenate(
            [w_attn[:, qcols], w_attn[:, kcols]], axis=1)
        b_qk = np.concatenate([b_attn[qcols], b_attn[kcols]])
        xT = np.ascontiguousarray(x[b].T)
        in_maps.append({
            "xT": xT.astype(BF16_NP),
            "w_qk": w_qk.astype(BF16_NP),
            "w_v": w_attn[:, vcols].astype(BF16_NP),
            "b_qk": b_qk[:, None].astype(np.float32),
            "b_v": b_attn[vcols][None, :].astype(BF16_NP),
            "w_proj": w_proj[qcols, :].astype(BF16_NP),
            "b_proj": (b_proj[None, :] if g == 0
                       else np.zeros((1, C), np.float32)).astype(BF16_NP),
            "mask": mask,
        })
    return in_maps


def run(x, w_attn, b_attn, w_proj, b_proj, trace=False):
    _install_ntff_hook()
    nc = build_nc()
    in_maps = make_in_maps(x, w_attn, b_attn, w_proj, b_proj)
    res = run_bass_kernel_spmd(nc, in_maps, list(range(N_CORES)), trace=trace)
    out = np.empty((B, T, C), dtype=np.float32)
    for i in range(N_CORES):
        b = i // TP
        r = i % TP
        o = res.results[i]["out"].astype(np.float32)
        for qt in range(4):
            out[b, 512 * qt + 128 * r:512 * qt + 128 * (r + 1), :] = \
                o[128 * qt:128 * (qt + 1)]
    return out, res


def kernel(x, w_attn, b_attn, w_proj, b_proj):
    out, _ = run(x, w_attn, b_attn, w_proj, b_proj, trace=False)
    return out
